# revision 16
# baseline (speedup 1.0000x reference)
"""Trainium2 Bass kernel for JoinAndSubsample (strided window gather).

reference semantics: x[B,T,D] -> edge-pad time by (3,3) -> out[B,TOUT,7*D]
where out[b,t,:] = concat(xp[b, 3t .. 3t+6, :]).  Since the 7 window frames
are consecutive, each output row is a contiguous 7*D-float slice of the
padded input starting at frame 3t.

Measured HW behaviour (this axon-tunneled trn2): DMA time is dominated by
a ~87 ns serialized cost PER DESCRIPTOR, nearly independent of descriptor
size.  A naive windowed store (one 2240 B descriptor per output row,
~10.9k descriptors/core) takes ~950 us.  So the kernel minimizes
descriptor count:

  - Loads stage x into SBUF, one contiguous run per (batch, chunk)
    partition (~148 descriptors, p = b*32 + chunk, 262 halo'd frames each).
  - The DVE (vector engine) gather-expands windows into a contiguous
    per-partition output region O (compute, no descriptors), half a chunk
    (43 rows) at a time -- I (83,840 B) + O (96,320 B) fit in a partition.
  - Stores write y from O as one contiguous run per partition per half
    (~256 descriptors total).

Per core (4 batches): HBM read 10.5 MB, write 24.5 MB (minimum possible),
~404 DMA descriptors instead of ~11k.
"""

from contextlib import ExitStack

import numpy as np

import concourse.bass as bass
import concourse.mybir as mybir
from concourse.ap import AP
from concourse.bass_utils import run_bass_kernel_spmd

LEFT, RIGHT, STRIDE, D = 3, 3, 3, 80
W = LEFT + RIGHT + 1            # 7 frames / window
B, T = 32, 8192
NCORES = 8
BPC = B // NCORES               # 4 batches per core
TOUT = (T - 1) // STRIDE + 1    # 2731
NCHUNK = 32                     # time-chunks per batch; BPC*NCHUNK = 128
MODE = "expand"


def build_nc(bpc=BPC, t=T, d=D, left=LEFT, right=RIGHT, stride=STRIDE,
             nchunk=NCHUNK, mode=MODE, sim_init=False, reps=1):
    """Build the per-core Bass module (parametric for small-scale sim tests).

    reps>1 repeats the whole sequence serially (cumulative semaphore
    targets) — used only for marginal-time benchmarking."""
    w = left + right + 1
    tout = (t - 1) // stride + 1
    nt = -(-tout // nchunk)                 # output rows per chunk (ceil)
    nt_last = tout - nt * (nchunk - 1)      # rows in last chunk
    assert nt_last >= 1
    fpc = stride * nt + (w - stride)        # frames per partition incl halo
    fpc_last = stride * nt_last + (w - stride)
    free = fpc * d                          # input-tile f32 elems / partition
    od = w * d                              # output row elems
    cl = nchunk - 1                         # last chunk index
    cl_start = cl * nt * stride - left      # first input frame of last chunk
    cl_cnt = t - cl_start                   # real frames available
    assert 0 < cl_cnt <= fpc_last
    n_rpad = fpc_last - cl_cnt              # right-pad frames to replicate
    # bulk load covers chunks 1..nchunk-2 entirely inside [0, t)
    assert (cl - 1) * nt * stride - left + fpc <= t
    assert nt * stride - left >= 0
    assert bpc * nchunk <= 128

    # expand mode: O region per partition holds ceil(nt/2) expanded rows
    hra = -(-nt // 2)                       # rows in half A
    hrb = nt - hra                          # rows in half B (regular chunks)
    hrb_last = nt_last - hra                # rows in half B, last chunk
    osz = hra * od                          # O elems per partition
    ps = free + osz if mode == "expand" else free   # partition stride

    # race detector is tensor-granular for DMA writes; our concurrent DMAs
    # write disjoint partitions/slots, so disable it (sim-only effect).
    nc = bass.Bass(detect_race_conditions=False)
    x = nc.declare_dram_parameter("x", [bpc, t, d], mybir.dt.float32,
                                  isOutput=False)
    y = nc.declare_dram_parameter("y", [bpc, tout, od], mybir.dt.float32,
                                  isOutput=True)

    with ExitStack() as ctx:
        tile = ctx.enter_context(
            nc.sbuf_tensor([bpc * nchunk, ps], mybir.dt.float32))
        psem = ctx.enter_context(nc.semaphore("pad_sem"))
        bsem = [ctx.enter_context(nc.semaphore(f"b{b}_sem"))
                for b in range(bpc)]
        ssem = ctx.enter_context(nc.semaphore("store_sem"))
        esem = ctx.enter_context(nc.semaphore("exp_sem"))
        isem = ctx.enter_context(nc.semaphore("init_sem"))
        block = ctx.enter_context(nc.Block())

        sb = tile[:].tensor
        n_pads = left + n_rpad
        npart = bpc * nchunk

        if sim_init:
            @block.vector
            def _(vector):
                vector.memset(tile[:], 0.0).then_inc(isem, 1)

        def issue_loads(eng, r=0, wait_stores=True, lsem=None):
            """13 DMAs: 5 pads (from HBM) + per batch bulk/chunk0/last.
            lsem: if given, ALL load DMAs inc it (expand mode);
            else pads inc psem and batch loads inc bsem[b]."""
            if sim_init and r == 0:
                eng.wait_ge(isem, 1)
            if r > 0 and wait_stores:
                # WAR: rep r's loads overwrite SBUF read by rep r-1's stores
                eng.wait_ge(ssem, (bpc + 1) * 16 * r)
            for k in range(left):
                eng.dma_start(
                    out=AP(sb, k * d, [[nchunk * ps, bpc], [1, d]]),
                    in_=AP(x, 0, [[t * d, bpc], [1, d]]),
                ).then_inc(lsem or psem, 16)
            for j in range(n_rpad):
                eng.dma_start(
                    out=AP(sb, cl * ps + (cl_cnt + j) * d,
                           [[nchunk * ps, bpc], [1, d]]),
                    in_=AP(x, (t - 1) * d, [[t * d, bpc], [1, d]]),
                ).then_inc(lsem or psem, 16)
            for b in range(bpc):
                eng.dma_start(
                    out=AP(sb, (b * nchunk + 1) * ps,
                           [[ps, nchunk - 2], [1, free]]),
                    in_=AP(x, b * t * d + (nt * stride - left) * d,
                           [[nt * stride * d, nchunk - 2], [1, free]]),
                ).then_inc(lsem or bsem[b], 16)
                eng.dma_start(
                    out=AP(sb, b * nchunk * ps + left * d,
                           [[ps, 1], [1, (fpc - left) * d]]),
                    in_=AP(x, b * t * d, [[t * d, 1], [1, (fpc - left) * d]]),
                ).then_inc(lsem or bsem[b], 16)
                eng.dma_start(
                    out=AP(sb, (b * nchunk + cl) * ps,
                           [[ps, 1], [1, cl_cnt * d]]),
                    in_=AP(x, b * t * d + cl_start * d,
                           [[t * d, 1], [1, cl_cnt * d]]),
                ).then_inc(lsem or bsem[b], 16)

        def issue_stores(eng, r=0):
            """Windowed (2240 B/descriptor) stores — overlap/serial modes."""
            eng.wait_ge(psem, n_pads * 16 * (r + 1))
            for b in range(bpc):
                eng.wait_ge(bsem[b], 3 * 16 * (r + 1))
                eng.dma_start(
                    out=AP(y, b * tout * od, [[nt * od, cl], [1, nt * od]]),
                    in_=AP(sb, b * nchunk * ps,
                           [[ps, cl], [stride * d, nt], [1, od]]),
                ).then_inc(ssem, 16)
            eng.dma_start(
                out=AP(y, cl * nt * od, [[tout * od, bpc], [1, nt_last * od]]),
                in_=AP(sb, cl * ps,
                       [[nchunk * ps, bpc], [stride * d, nt_last], [1, od]]),
            ).then_inc(ssem, 16)
            eng.wait_ge(ssem, (bpc + 1) * 16 * (r + 1))

        if mode == "expand":
            assert hrb >= 1 and 0 <= hrb_last <= hrb
            n_loads = n_pads + 3 * bpc           # 13
            n_st = 2 * bpc + (1 if hrb_last > 0 else 0)   # store DMAs/rep
            lsem = psem                          # single load sem

            @block.sync
            def _(sync):
                for r in range(reps):
                    if r > 0:
                        # loads overwrite I, last read by half-B expansion
                        sync.wait_ge(esem, 2 * r)
                    issue_loads(sync, r, wait_stores=False, lsem=lsem)

            @block.vector
            def _(vector):
                if sim_init:
                    vector.wait_ge(isem, 1)
                for r in range(reps):
                    # half A: O[q,:] = I[3q+j,:], q<hra -- all partitions
                    vector.wait_ge(lsem, n_loads * 16 * (r + 1))
                    if r > 0:   # O still being read by rep r-1 half-B store
                        vector.wait_ge(ssem, n_st * 16 * r)
                    vector.tensor_copy(
                        out=AP(sb, free, [[ps, npart], [od, hra], [1, od]]),
                        in_=AP(sb, 0, [[ps, npart], [stride * d, hra],
                                       [1, od]]),
                    ).then_inc(esem, 1)
                    # half B: O[q,:] = I[3*(hra+q)+j,:] -- garbage rows for
                    # the last chunk beyond hrb_last are computed but never
                    # stored (reads stay inside the 262-frame allocation)
                    vector.wait_ge(ssem, (n_st * r + bpc) * 16)  # A stored
                    vector.tensor_copy(
                        out=AP(sb, free, [[ps, npart], [od, hra], [1, od]]),
                        in_=AP(sb, hra * stride * d,
                               [[ps, npart], [stride * d, hra], [1, od]]),
                    ).then_inc(esem, 1)

            @block.scalar
            def _(scalar):
                for r in range(reps):
                    # half A stores: rows [0, hra) of every chunk incl. last
                    scalar.wait_ge(esem, 2 * r + 1)
                    for b in range(bpc):
                        scalar.dma_start(
                            out=AP(y, b * tout * od,
                                   [[nt * od, nchunk], [1, hra * od]]),
                            in_=AP(sb, b * nchunk * ps + free,
                                   [[ps, nchunk], [1, hra * od]]),
                        ).then_inc(ssem, 16)
                    # half B stores: rows [hra, nt) of chunks 0..cl-1
                    scalar.wait_ge(esem, 2 * r + 2)
                    for b in range(bpc):
                        scalar.dma_start(
                            out=AP(y, b * tout * od + hra * od,
                                   [[nt * od, cl], [1, hrb * od]]),
                            in_=AP(sb, b * nchunk * ps + free,
                                   [[ps, cl], [1, hrb * od]]),
                        ).then_inc(ssem, 16)
                    # half B tail: rows [hra, nt_last) of the last chunk
                    if hrb_last > 0:
                        scalar.dma_start(
                            out=AP(y, (cl * nt + hra) * od,
                                   [[tout * od, bpc], [1, hrb_last * od]]),
                            in_=AP(sb, cl * ps + free,
                                   [[nchunk * ps, bpc], [1, hrb_last * od]]),
                        ).then_inc(ssem, 16)
                    scalar.wait_ge(ssem, n_st * 16 * (r + 1))

        elif mode == "overlap":
            @block.sync
            def _(sync):
                for r in range(reps):
                    issue_loads(sync, r)

            @block.scalar
            def _(scalar):
                for r in range(reps):
                    issue_stores(scalar, r)
        elif mode == "serial":
            @block.sync
            def _(sync):
                for r in range(reps):
                    issue_loads(sync, r)
                    issue_stores(sync, r)
        elif mode == "loadonly":     # diagnostic: loads only, y stays 0
            @block.sync
            def _(sync):
                for r in range(reps):
                    if r > 0:      # serialize reps on load completion
                        sync.wait_ge(psem, n_pads * 16 * r)
                        for b in range(bpc):
                            sync.wait_ge(bsem[b], 3 * 16 * r)
                    issue_loads(sync, r, wait_stores=False)
                sync.wait_ge(psem, n_pads * 16 * reps)
                for b in range(bpc):
                    sync.wait_ge(bsem[b], 3 * 16 * reps)
        elif mode == "storeonly":    # diagnostic: stores of uninit SBUF
            @block.scalar
            def _(scalar):
                for r in range(reps):
                    for b in range(bpc):
                        scalar.dma_start(
                            out=AP(y, b * tout * od,
                                   [[nt * od, cl], [1, nt * od]]),
                            in_=AP(sb, b * nchunk * ps,
                                   [[ps, cl], [stride * d, nt], [1, od]]),
                        ).then_inc(ssem, 16)
                    scalar.dma_start(
                        out=AP(y, cl * nt * od,
                               [[tout * od, bpc], [1, nt_last * od]]),
                        in_=AP(sb, cl * ps,
                               [[nchunk * ps, bpc], [stride * d, nt_last],
                                [1, od]]),
                    ).then_inc(ssem, 16)
                    scalar.wait_ge(ssem, (bpc + 1) * 16 * (r + 1))
        elif mode == "storesplit":   # diagnostic: windowed store 3-way split
            def batch_store(eng, b):
                eng.dma_start(
                    out=AP(y, b * tout * od, [[nt * od, cl], [1, nt * od]]),
                    in_=AP(sb, b * nchunk * ps,
                           [[ps, cl], [stride * d, nt], [1, od]]),
                ).then_inc(ssem, 16)

            @block.sync
            def _(sync):
                for r in range(reps):
                    if r > 0:
                        sync.wait_ge(ssem, 4 * 16 * r)
                    batch_store(sync, 0)

            @block.scalar
            def _(scalar):
                for r in range(reps):
                    if r > 0:
                        scalar.wait_ge(ssem, 4 * 16 * r)
                    batch_store(scalar, 1)

            @block.gpsimd
            def _(gp):
                for r in range(reps):
                    if r > 0:
                        gp.wait_ge(ssem, 4 * 16 * r)
                    batch_store(gp, 2)
                    batch_store(gp, 3)
                gp.wait_ge(ssem, 4 * 16 * reps)
        elif mode in ("exponly", "expcontig", "expsmall", "expgp"):
            # diagnostics: expansion copies only (no DMA)
            if mode == "exponly":      # the real strided gather copy x2
                shp = [[stride * d, hra], [1, od]]
                n_copies, src_off = 2, hra * stride * d
            elif mode == "expcontig":  # same size, contiguous src x2
                shp = [[od, hra], [1, od]]
                n_copies, src_off = 2, 0
            elif mode == "expsmall":   # tiny copy x2 (fixed overhead)
                shp = [[1 * od, 1], [1, od]]
                n_copies, src_off = 2, 0
            else:                      # expgp: strided copy on gpsimd
                shp = [[stride * d, hra], [1, od]]
                n_copies, src_off = 2, hra * stride * d

            eng_sel = "gpsimd" if mode == "expgp" else "vector"

            def body(eng):
                for r in range(reps):
                    for c in range(n_copies):
                        eng.tensor_copy(
                            out=AP(sb, free, [[ps, npart]] + shp),
                            in_=AP(sb, src_off * (c % 2), [[ps, npart]] + shp),
                        ).then_inc(esem, 1)
                eng.wait_ge(esem, n_copies * reps)

            if eng_sel == "vector":
                @block.vector
                def _(vector):
                    body(vector)
            else:
                @block.gpsimd
                def _(gp):
                    body(gp)
        elif mode == "storecontig":  # diagnostic: pure store BW, 128 descs
            @block.scalar
            def _(scalar):
                for r in range(reps):
                    for k in range(2):   # 2 x 10.7 MB ~ output size
                        scalar.dma_start(
                            out=AP(y, k * npart * free,
                                   [[free, npart], [1, free]]),
                            in_=AP(sb, 0, [[ps, npart], [1, free]]),
                        ).then_inc(ssem, 16)
                    scalar.wait_ge(ssem, 2 * 16 * (r + 1))
        elif mode == "loadcontig":   # diagnostic: pure load BW, 128 descs
            per = (bpc * t * d) // npart
            assert per <= free
            @block.sync
            def _(sync):
                for r in range(reps):
                    sync.dma_start(
                        out=AP(sb, 0, [[ps, npart], [1, per]]),
                        in_=AP(x, 0, [[per, npart], [1, per]]),
                    ).then_inc(psem, 16)
                    sync.wait_ge(psem, 16 * (r + 1))
        else:
            raise ValueError(mode)

    return nc


_NC = None


def _get_nc():
    global _NC
    if _NC is None:
        _NC = build_nc()
    return _NC


def kernel(**inputs):
    x = np.ascontiguousarray(inputs["x"], dtype=np.float32)
    assert x.shape == (B, T, D)
    nc = _get_nc()
    in_maps = [{"x": x[i * BPC:(i + 1) * BPC]} for i in range(NCORES)]
    res = run_bass_kernel_spmd(nc, in_maps, list(range(NCORES)))
    return np.concatenate([res.results[i]["y"] for i in range(NCORES)], axis=0)


# revision 19
# speedup vs baseline: 1.0340x; 1.0340x over previous
"""Trainium2 Bass kernel for JoinAndSubsample (strided window gather).

reference semantics: x[B,T,D] -> edge-pad time by (3,3) -> out[B,TOUT,7*D]
where out[b,t,:] = concat(xp[b, 3t .. 3t+6, :]).  Since the 7 window frames
are consecutive, each output row is a contiguous 7*D-float slice of the
padded input starting at frame 3t.

Measured HW behaviour (this axon-tunneled trn2): DMA time is dominated by
a ~87 ns serialized cost PER DESCRIPTOR, nearly independent of descriptor
size.  A naive windowed store (one 2240 B descriptor per output row,
~10.9k descriptors/core) takes ~950 us.  So the kernel minimizes
descriptor count:

  - Loads stage x into SBUF, one contiguous run per (batch, chunk)
    partition (~148 descriptors, p = b*32 + chunk, 262 halo'd frames each).
  - The DVE (vector engine) gather-expands windows into a contiguous
    per-partition output region O (compute, no descriptors), half a chunk
    (43 rows) at a time -- I (83,840 B) + O (96,320 B) fit in a partition.
  - Stores write y from O as one contiguous run per partition per half
    (~256 descriptors total).

Per core (4 batches): HBM read 10.5 MB, write 24.5 MB (minimum possible),
~404 DMA descriptors instead of ~11k.
"""

from contextlib import ExitStack

import numpy as np

import concourse.bass as bass
import concourse.mybir as mybir
from concourse.ap import AP
from concourse.bass_utils import run_bass_kernel_spmd

LEFT, RIGHT, STRIDE, D = 3, 3, 3, 80
W = LEFT + RIGHT + 1            # 7 frames / window
B, T = 32, 8192
NCORES = 8
BPC = B // NCORES               # 4 batches per core
TOUT = (T - 1) // STRIDE + 1    # 2731
NCHUNK = 32                     # time-chunks per batch; BPC*NCHUNK = 128
MODE = "expand"


def build_nc(bpc=BPC, t=T, d=D, left=LEFT, right=RIGHT, stride=STRIDE,
             nchunk=NCHUNK, mode=MODE, sim_init=False, reps=1):
    """Build the per-core Bass module (parametric for small-scale sim tests).

    reps>1 repeats the whole sequence serially (cumulative semaphore
    targets) — used only for marginal-time benchmarking."""
    w = left + right + 1
    tout = (t - 1) // stride + 1
    nt = -(-tout // nchunk)                 # output rows per chunk (ceil)
    nt_last = tout - nt * (nchunk - 1)      # rows in last chunk
    assert nt_last >= 1
    fpc = stride * nt + (w - stride)        # frames per partition incl halo
    fpc_last = stride * nt_last + (w - stride)
    free = fpc * d                          # input-tile f32 elems / partition
    od = w * d                              # output row elems
    cl = nchunk - 1                         # last chunk index
    cl_start = cl * nt * stride - left      # first input frame of last chunk
    cl_cnt = t - cl_start                   # real frames available
    assert 0 < cl_cnt <= fpc_last
    n_rpad = fpc_last - cl_cnt              # right-pad frames to replicate
    # bulk load covers chunks 1..nchunk-2 entirely inside [0, t)
    assert (cl - 1) * nt * stride - left + fpc <= t
    assert nt * stride - left >= 0
    assert bpc * nchunk <= 128

    # expand mode: O region per partition holds ceil(nt/2) expanded rows
    hra = -(-nt // 2)                       # rows in half A
    hrb = nt - hra                          # rows in half B (regular chunks)
    hrb_last = nt_last - hra                # rows in half B, last chunk
    osz = hra * od                          # O elems per partition
    has_o = mode.startswith("exp")          # modes with the O region
    ps = free + osz if has_o else free      # partition stride

    # race detector is tensor-granular for DMA writes; our concurrent DMAs
    # write disjoint partitions/slots, so disable it (sim-only effect).
    nc = bass.Bass(detect_race_conditions=False)
    x = nc.declare_dram_parameter("x", [bpc, t, d], mybir.dt.float32,
                                  isOutput=False)
    y = nc.declare_dram_parameter("y", [bpc, tout, od], mybir.dt.float32,
                                  isOutput=True)

    with ExitStack() as ctx:
        tile = ctx.enter_context(
            nc.sbuf_tensor([bpc * nchunk, ps], mybir.dt.float32))
        psem = ctx.enter_context(nc.semaphore("pad_sem"))
        bsem = [ctx.enter_context(nc.semaphore(f"b{b}_sem"))
                for b in range(bpc)]
        ssem = ctx.enter_context(nc.semaphore("store_sem"))
        esem = ctx.enter_context(nc.semaphore("exp_sem"))
        isem = ctx.enter_context(nc.semaphore("init_sem"))
        block = ctx.enter_context(nc.Block())

        sb = tile[:].tensor
        n_pads = left + n_rpad
        npart = bpc * nchunk

        if sim_init:
            @block.vector
            def _(vector):
                vector.memset(tile[:], 0.0).then_inc(isem, 1)

        def issue_loads(eng, r=0, wait_stores=True, lsem=None):
            """13 DMAs: 5 pads (from HBM) + per batch bulk/chunk0/last.
            lsem: if given, ALL load DMAs inc it (expand mode);
            else pads inc psem and batch loads inc bsem[b]."""
            if sim_init and r == 0:
                eng.wait_ge(isem, 1)
            if r > 0 and wait_stores:
                # WAR: rep r's loads overwrite SBUF read by rep r-1's stores
                eng.wait_ge(ssem, (bpc + 1) * 16 * r)
            for k in range(left):
                eng.dma_start(
                    out=AP(sb, k * d, [[nchunk * ps, bpc], [1, d]]),
                    in_=AP(x, 0, [[t * d, bpc], [1, d]]),
                ).then_inc(lsem or psem, 16)
            for j in range(n_rpad):
                eng.dma_start(
                    out=AP(sb, cl * ps + (cl_cnt + j) * d,
                           [[nchunk * ps, bpc], [1, d]]),
                    in_=AP(x, (t - 1) * d, [[t * d, bpc], [1, d]]),
                ).then_inc(lsem or psem, 16)
            for b in range(bpc):
                eng.dma_start(
                    out=AP(sb, (b * nchunk + 1) * ps,
                           [[ps, nchunk - 2], [1, free]]),
                    in_=AP(x, b * t * d + (nt * stride - left) * d,
                           [[nt * stride * d, nchunk - 2], [1, free]]),
                ).then_inc(lsem or bsem[b], 16)
                eng.dma_start(
                    out=AP(sb, b * nchunk * ps + left * d,
                           [[ps, 1], [1, (fpc - left) * d]]),
                    in_=AP(x, b * t * d, [[t * d, 1], [1, (fpc - left) * d]]),
                ).then_inc(lsem or bsem[b], 16)
                eng.dma_start(
                    out=AP(sb, (b * nchunk + cl) * ps,
                           [[ps, 1], [1, cl_cnt * d]]),
                    in_=AP(x, b * t * d + cl_start * d,
                           [[t * d, 1], [1, cl_cnt * d]]),
                ).then_inc(lsem or bsem[b], 16)

        def issue_stores(eng, r=0):
            """Windowed (2240 B/descriptor) stores — overlap/serial modes."""
            eng.wait_ge(psem, n_pads * 16 * (r + 1))
            for b in range(bpc):
                eng.wait_ge(bsem[b], 3 * 16 * (r + 1))
                eng.dma_start(
                    out=AP(y, b * tout * od, [[nt * od, cl], [1, nt * od]]),
                    in_=AP(sb, b * nchunk * ps,
                           [[ps, cl], [stride * d, nt], [1, od]]),
                ).then_inc(ssem, 16)
            eng.dma_start(
                out=AP(y, cl * nt * od, [[tout * od, bpc], [1, nt_last * od]]),
                in_=AP(sb, cl * ps,
                       [[nchunk * ps, bpc], [stride * d, nt_last], [1, od]]),
            ).then_inc(ssem, 16)
            eng.wait_ge(ssem, (bpc + 1) * 16 * (r + 1))

        if mode == "expand":
            assert hrb >= 1 and 0 <= hrb_last <= hrb
            n_loads = n_pads + 3 * bpc           # 13
            n_st = 2 * bpc + (1 if hrb_last > 0 else 0)   # store DMAs/rep
            lsem = psem                          # single load sem

            @block.sync
            def _(sync):
                for r in range(reps):
                    if r > 0:
                        # loads overwrite I, last read by half-B expansion
                        sync.wait_ge(esem, 2 * r)
                    issue_loads(sync, r, wait_stores=False, lsem=lsem)

            @block.vector
            def _(vector):
                if sim_init:
                    vector.wait_ge(isem, 1)
                for r in range(reps):
                    # half A: O[q,:] = I[3q+j,:], q<hra -- all partitions
                    vector.wait_ge(lsem, n_loads * 16 * (r + 1))
                    if r > 0:   # O still being read by rep r-1 half-B store
                        vector.wait_ge(ssem, n_st * 16 * r)
                    vector.tensor_copy(
                        out=AP(sb, free, [[ps, npart], [od, hra], [1, od]]),
                        in_=AP(sb, 0, [[ps, npart], [stride * d, hra],
                                       [1, od]]),
                    ).then_inc(esem, 1)
                    # half B: O[q,:] = I[3*(hra+q)+j,:] -- garbage rows for
                    # the last chunk beyond hrb_last are computed but never
                    # stored (reads stay inside the 262-frame allocation)
                    vector.wait_ge(ssem, (n_st * r + bpc) * 16)  # A stored
                    vector.tensor_copy(
                        out=AP(sb, free, [[ps, npart], [od, hra], [1, od]]),
                        in_=AP(sb, hra * stride * d,
                               [[ps, npart], [stride * d, hra], [1, od]]),
                    ).then_inc(esem, 1)

            @block.scalar
            def _(scalar):
                for r in range(reps):
                    # half A stores: rows [0, hra) of every chunk incl. last
                    scalar.wait_ge(esem, 2 * r + 1)
                    for b in range(bpc):
                        scalar.dma_start(
                            out=AP(y, b * tout * od,
                                   [[nt * od, nchunk], [1, hra * od]]),
                            in_=AP(sb, b * nchunk * ps + free,
                                   [[ps, nchunk], [1, hra * od]]),
                        ).then_inc(ssem, 16)
                    # half B stores: rows [hra, nt) of chunks 0..cl-1
                    scalar.wait_ge(esem, 2 * r + 2)
                    for b in range(bpc):
                        scalar.dma_start(
                            out=AP(y, b * tout * od + hra * od,
                                   [[nt * od, cl], [1, hrb * od]]),
                            in_=AP(sb, b * nchunk * ps + free,
                                   [[ps, cl], [1, hrb * od]]),
                        ).then_inc(ssem, 16)
                    # half B tail: rows [hra, nt_last) of the last chunk
                    if hrb_last > 0:
                        scalar.dma_start(
                            out=AP(y, (cl * nt + hra) * od,
                                   [[tout * od, bpc], [1, hrb_last * od]]),
                            in_=AP(sb, cl * ps + free,
                                   [[nchunk * ps, bpc], [1, hrb_last * od]]),
                        ).then_inc(ssem, 16)
                    scalar.wait_ge(ssem, n_st * 16 * (r + 1))

        elif mode == "overlap":
            @block.sync
            def _(sync):
                for r in range(reps):
                    issue_loads(sync, r)

            @block.scalar
            def _(scalar):
                for r in range(reps):
                    issue_stores(scalar, r)
        elif mode == "serial":
            @block.sync
            def _(sync):
                for r in range(reps):
                    issue_loads(sync, r)
                    issue_stores(sync, r)
        elif mode == "loadonly":     # diagnostic: loads only, y stays 0
            @block.sync
            def _(sync):
                for r in range(reps):
                    if r > 0:      # serialize reps on load completion
                        sync.wait_ge(psem, n_pads * 16 * r)
                        for b in range(bpc):
                            sync.wait_ge(bsem[b], 3 * 16 * r)
                    issue_loads(sync, r, wait_stores=False)
                sync.wait_ge(psem, n_pads * 16 * reps)
                for b in range(bpc):
                    sync.wait_ge(bsem[b], 3 * 16 * reps)
        elif mode == "storeonly":    # diagnostic: stores of uninit SBUF
            @block.scalar
            def _(scalar):
                for r in range(reps):
                    for b in range(bpc):
                        scalar.dma_start(
                            out=AP(y, b * tout * od,
                                   [[nt * od, cl], [1, nt * od]]),
                            in_=AP(sb, b * nchunk * ps,
                                   [[ps, cl], [stride * d, nt], [1, od]]),
                        ).then_inc(ssem, 16)
                    scalar.dma_start(
                        out=AP(y, cl * nt * od,
                               [[tout * od, bpc], [1, nt_last * od]]),
                        in_=AP(sb, cl * ps,
                               [[nchunk * ps, bpc], [stride * d, nt_last],
                                [1, od]]),
                    ).then_inc(ssem, 16)
                    scalar.wait_ge(ssem, (bpc + 1) * 16 * (r + 1))
        elif mode == "storesplit2":  # diagnostic: windowed store 2-way split
            def batch_store2(eng, b):
                eng.dma_start(
                    out=AP(y, b * tout * od, [[nt * od, cl], [1, nt * od]]),
                    in_=AP(sb, b * nchunk * ps,
                           [[ps, cl], [stride * d, nt], [1, od]]),
                ).then_inc(ssem, 16)

            @block.sync
            def _(sync):
                for r in range(reps):
                    if r > 0:
                        sync.wait_ge(ssem, 4 * 16 * r)
                    batch_store2(sync, 0)
                    batch_store2(sync, 1)
                sync.wait_ge(ssem, 4 * 16 * reps)

            @block.scalar
            def _(scalar):
                for r in range(reps):
                    if r > 0:
                        scalar.wait_ge(ssem, 4 * 16 * r)
                    batch_store2(scalar, 2)
                    batch_store2(scalar, 3)
                scalar.wait_ge(ssem, 4 * 16 * reps)
        elif mode == "storesplit":   # diagnostic: windowed store 3-way split
            def batch_store(eng, b):
                eng.dma_start(
                    out=AP(y, b * tout * od, [[nt * od, cl], [1, nt * od]]),
                    in_=AP(sb, b * nchunk * ps,
                           [[ps, cl], [stride * d, nt], [1, od]]),
                ).then_inc(ssem, 16)

            @block.sync
            def _(sync):
                for r in range(reps):
                    if r > 0:
                        sync.wait_ge(ssem, 4 * 16 * r)
                    batch_store(sync, 0)

            @block.scalar
            def _(scalar):
                for r in range(reps):
                    if r > 0:
                        scalar.wait_ge(ssem, 4 * 16 * r)
                    batch_store(scalar, 1)

            @block.gpsimd
            def _(gp):
                for r in range(reps):
                    if r > 0:
                        gp.wait_ge(ssem, 4 * 16 * r)
                    batch_store(gp, 2)
                    batch_store(gp, 3)
                gp.wait_ge(ssem, 4 * 16 * reps)
        elif mode in ("exponly", "expcontig", "expsmall", "expgp"):
            # diagnostics: expansion copies only (no DMA); out is always
            # the contiguous O region, in varies
            if mode in ("exponly", "expgp"):   # the real strided gather copy
                in_shp = [[stride * d, hra], [1, od]]
                rows, src_off = hra, hra * stride * d
            elif mode == "expcontig":          # same size, contiguous src
                in_shp = [[od, hra], [1, od]]
                rows, src_off = hra, 0
            else:                              # expsmall: 1-row copy
                in_shp = [[od, 1], [1, od]]
                rows, src_off = 1, 0

            eng_sel = "gpsimd" if mode == "expgp" else "vector"

            def body(eng):
                for r in range(reps):
                    for c in range(2):
                        eng.tensor_copy(
                            out=AP(sb, free,
                                   [[ps, npart], [od, rows], [1, od]]),
                            in_=AP(sb, src_off * (c % 2),
                                   [[ps, npart]] + in_shp),
                        ).then_inc(esem, 1)
                eng.wait_ge(esem, 2 * reps)

            if eng_sel == "vector":
                @block.vector
                def _(vector):
                    body(vector)
            else:
                @block.gpsimd
                def _(gp):
                    body(gp)
        elif mode == "storecontig":  # diagnostic: pure store BW, 128 descs
            @block.scalar
            def _(scalar):
                for r in range(reps):
                    for k in range(2):   # 2 x 10.7 MB ~ output size
                        scalar.dma_start(
                            out=AP(y, k * npart * free,
                                   [[free, npart], [1, free]]),
                            in_=AP(sb, 0, [[ps, npart], [1, free]]),
                        ).then_inc(ssem, 16)
                    scalar.wait_ge(ssem, 2 * 16 * (r + 1))
        elif mode == "loadcontig":   # diagnostic: pure load BW, 128 descs
            per = (bpc * t * d) // npart
            assert per <= free
            @block.sync
            def _(sync):
                for r in range(reps):
                    sync.dma_start(
                        out=AP(sb, 0, [[ps, npart], [1, per]]),
                        in_=AP(x, 0, [[per, npart], [1, per]]),
                    ).then_inc(psem, 16)
                    sync.wait_ge(psem, 16 * (r + 1))
        else:
            raise ValueError(mode)

    return nc


_NC = None


def _get_nc():
    global _NC
    if _NC is None:
        _NC = build_nc()
    return _NC


def kernel(**inputs):
    x = np.ascontiguousarray(inputs["x"], dtype=np.float32)
    assert x.shape == (B, T, D)
    nc = _get_nc()
    in_maps = [{"x": x[i * BPC:(i + 1) * BPC]} for i in range(NCORES)]
    res = run_bass_kernel_spmd(nc, in_maps, list(range(NCORES)))
    return np.concatenate([res.results[i]["y"] for i in range(NCORES)], axis=0)
